# revision 1
# baseline (speedup 1.0000x reference)
"""Trainium2 Bass kernel for nn_MultiHeadAttention (B=4, S=2048, H=16, D=64, E=1024).

Sharding: 8 cores = 4 batches x 2 head-groups (8 heads each). Each core gets
its batch's x (transposed on host) and its head-group's slices of Wq/Wk/Wv/sqk
(transposed; q/k columns permuted so each head's RoPE pair-components are
contiguous halves), and produces the [S, 512] slice of the output; the host
concatenates slices.

Per-core pipeline (self-measured ~0.42-0.48 ms on HW; CoreSim model 424 us):
  phase A (~130 us, DVE-bound):
    - q|k projections into one fused [128, 1024] PSUM tile + v projection,
      f32r matmuls (1 cycle/row at N=512, vs 4 for fp32);
    - sum-of-squares for the L2 norm taken from the PRE-RoPE values (rotations
      preserve norms) via ScalarE Square + DVE per-head reduce, so it runs in
      parallel with RoPE;
    - RoPE on DVE in bf16 (2x mode; 6 tensor ops per tile over both tensors);
    - rsqrt via 0x5f3759df bit-trick + 2 Newton steps, batched [128, 64];
    - normalize (DVE tensor_scalar, bf16) and PE-transpose to [d, s] bf16;
      sqk^2*(1/base_scale)^2 is folded into k-hat on the transpose
      evacuation (ScalarE Copy with per-partition scale).
  phase B (~293 us, ScalarE-bound -- the hard floor is 33.5M exps/core at
  128 lanes * 1.2 GHz = 218 us):
    - per head: scores_T[k, q] = k-hat^T q-hat as bf16 matmuls (K=64) into
      [128, 1024] PSUM tiles, double-buffered so ScalarE never waits;
    - exp(scores/8) on ScalarE, PSUM -> SBUF (scale folded into the
      activation), f32r output;
    - PV plus the softmax denominator in one f32r matmul per (kt, q-block)
      using a ones-augmented V (lhsT [128 x 65], accumulated over 16 k-tiles);
    - PE transpose of the [65, 512] accumulators, DVE reciprocal of the
      denominator column, tensor_scalar normalize, one 2 KB-row DMA per
      128 output rows.

No collectives: softmax rows live entirely on one core by construction.
"""

import os
import sys

import numpy as np

B, S, H, D, E = 4, 2048, 16, 64, 1024
NCORES = 8
HL = 8          # heads per core
O = HL * D      # 512 per-core projection width
P = 128
ECH = E // P    # 8 contraction chunks
NBT = S // P    # 16 bs tiles
NBLK = 4        # bs blocks of 512
NKT = S // P    # 16 k tiles
OT = O // P     # 4 o tiles

_MAGIC_P1 = 0x5F3759DF + 1

_built = None


def _ensure_paths():
    for p in ("/opt/trn_rl_repo",):
        if os.path.isdir(p) and p not in sys.path:
            sys.path.insert(0, p)


def _install_walrus_compat():
    """This container's walrus accepts at most ONE sem wait per instruction.
    Split multi-wait instructions into single-wait NoOps in the BIR JSON just
    before compilation."""
    import json

    from concourse import bass2jax, bass_utils

    if getattr(bass2jax.compile_bir_kernel, "_single_wait_legal", False):
        return

    orig = bass_utils.compile_bir_kernel

    def _legalize(bir_json: bytes) -> bytes:
        d = json.loads(bir_json)
        ctr = 0
        for fn in d["functions"]:
            for bb in fn["blocks"]:
                out = []
                for inst in bb["instructions"]:
                    si = inst.get("sync_info")
                    waits = si.get("on_wait") if si else None
                    if waits and len(waits) > 1:
                        for w in waits[:-1]:
                            ctr += 1
                            nop = {
                                "engine": inst["engine"],
                                "ins": [],
                                "outs": [],
                                "name": f"I-wsplit-{ctr}",
                                "opcode": "NoOp",
                                "sync_info": {"on_update": [], "on_wait": [w]},
                            }
                            if inst.get("debug") is not None:
                                nop["debug"] = inst["debug"]
                            out.append(nop)
                        si["on_wait"] = [waits[-1]]
                    out.append(inst)
                bb["instructions"] = out
        return json.dumps(d).encode()

    def wrapper(bir_json, tmpdir, neff_name="file.neff"):
        return orig(_legalize(bir_json), tmpdir, neff_name)

    wrapper._single_wait_legal = True
    bass2jax.compile_bir_kernel = wrapper


def _install_drain_patch():
    """Same walrus limitation applies to the TileContext final drain: spread
    its sem waits over single-wait NoOps."""
    import bass_rust
    import concourse.tile as tile
    from concourse.vector_clock import ScopedClock

    if getattr(tile.TileContext._drain_and_barrier, "_single_wait", False):
        return

    def _patched(self, tick_clock, wait_clock):
        nc = self.nc
        drain_inst = nc.sync.drain()
        wait_clock.add_sem_waits(
            drain_inst.ins, ScopedClock({None: tick_clock.global_clock})
        )
        waits = list(drain_inst.ins.sync_info.on_wait)
        if len(waits) > 1:
            drain_inst.ins.sync_info.on_wait.clear()
            drain_inst.ins.sync_info.on_wait.extend(waits[:1])
            for w in waits[1:]:
                nop = nc.sync.nop(nofuse=True)
                nop.ins.sync_info = bass_rust.SyncInfo(on_wait=[w], on_update=[])
        nc.all_engine_barrier()
        assert self.sems is not None
        popped = nc._tile_sem_poison_stack.pop()
        assert popped is self._sem_poison
        nc.clear_and_free_semaphores(list(self.sems.allocated().values()))
        nc.all_engine_barrier()

    _patched._single_wait = True
    tile.TileContext._drain_and_barrier = _patched


def build_program(use_f32r=True, use_bf16_scores=True, repeat=1, phases="ab"):
    """Build the per-core Bass/Tile program (identical on all cores)."""
    _ensure_paths()
    _install_walrus_compat()
    _install_drain_patch()

    import concourse.bass as bass
    import concourse.tile as tile
    from concourse import mybir
    from concourse.masks import make_identity

    f32 = mybir.dt.float32
    bf16 = mybir.dt.bfloat16
    f32r = mybir.dt.float32r
    i32 = mybir.dt.int32
    ALU = mybir.AluOpType

    mm_dt = f32r if use_f32r else f32
    sc_dt = bf16 if use_bf16_scores else f32

    def mmcast(ap):
        return ap.bitcast(mm_dt) if use_f32r else ap

    nc = bass.Bass("TRN2", target_bir_lowering=False, debug=False)

    xT = nc.dram_tensor("xT", [E, S], mm_dt, kind="ExternalInput")
    wqT = nc.dram_tensor("wqT", [E, O], mm_dt, kind="ExternalInput")
    wkT = nc.dram_tensor("wkT", [E, O], mm_dt, kind="ExternalInput")
    wvT = nc.dram_tensor("wvT", [E, O], mm_dt, kind="ExternalInput")
    s2p = nc.dram_tensor("s2p", [OT, P, 1], f32, kind="ExternalInput")
    cos16 = nc.dram_tensor("cos16", [S, O], bf16, kind="ExternalInput")
    sin16 = nc.dram_tensor("sin16", [S, O], bf16, kind="ExternalInput")
    out = nc.dram_tensor("out", [S, O], f32, kind="ExternalOutput")

    from contextlib import ExitStack

    with tile.TileContext(nc) as tc, ExitStack() as ctx:
        # ---------------- persistent tiles ----------------
        pp = ctx.enter_context(tc.tile_pool(name="persist", bufs=1))
        qT = [pp.tile([P, S], sc_dt, name=f"qT{i}", tag=f"qT{i}") for i in range(OT)]
        kT = [pp.tile([P, S], sc_dt, name=f"kT{i}", tag=f"kT{i}") for i in range(OT)]
        vo = [pp.tile([P, HL, D + 1], mm_dt, name=f"vo{i}", tag=f"vo{i}") for i in range(NBT)]
        out_sb = [pp.tile([P, O], f32, name=f"osb{i}", tag=f"osb{i}") for i in range(NBT)]
        ident_b = pp.tile([P, P], sc_dt, name="ident_b", tag="ident_b")
        make_identity(nc, ident_b)
        ident_f = pp.tile([P, P], f32, name="ident_f", tag="ident_f")
        make_identity(nc, ident_f)
        s2c = [pp.tile([P, 1], f32, name=f"s2c{i}", tag=f"s2c{i}") for i in range(OT)]
        for i in range(OT):
            nc.sync.dma_start(out=s2c[i], in_=s2p[i])

        for _rep in range(repeat):
            # ================= phase A: projections + rope + norm =================
            if "a" not in phases:
                pass
            else:
              with tc.tile_pool(name="pa", bufs=1) as pa, tc.tile_pool(
                name="psA", bufs=1, space="PSUM"
            ) as psA:
                wq = []
                wk = []
                wv = []
                for ec in range(ECH):
                    tq = pa.tile([P, O], mm_dt, name=f"wq{ec}", tag=f"wq{ec}")
                    nc.sync.dma_start(out=tq, in_=wqT[ec * P : (ec + 1) * P, :])
                    wq.append(tq)
                    tk = pa.tile([P, O], mm_dt, name=f"wk{ec}", tag=f"wk{ec}")
                    nc.sync.dma_start(out=tk, in_=wkT[ec * P : (ec + 1) * P, :])
                    wk.append(tk)
                    tv = pa.tile([P, O], mm_dt, name=f"wv{ec}", tag=f"wv{ec}")
                    nc.sync.dma_start(out=tv, in_=wvT[ec * P : (ec + 1) * P, :])
                    wv.append(tv)

                for blk in range(NBLK):
                    xts = []
                    for ec in range(ECH):
                        xt = pa.tile([P, 512], mm_dt, tag=f"xt{ec}", bufs=2 if ec < 4 else 1, name=f"xt{ec}")
                        nc.sync.dma_start(
                            out=xt, in_=xT[ec * P : (ec + 1) * P, blk * 512 : (blk + 1) * 512]
                        )
                        xts.append(xt)

                    # [p, t, (q|k), head] sums of squares for the block
                    ssq = pa.tile([P, 4, 2, HL], f32, tag="ssq", bufs=2, name="ssq")
                    qkrs = []
                    for t in range(4):
                        bst = blk * 4 + t
                        s0 = bst * P
                        cos_t = pa.tile([P, 2, HL, 32], bf16, tag="cos", bufs=2, name="cos_t")
                        nc.sync.dma_start(out=cos_t, in_=cos16[s0 : s0 + P, :])
                        sin_t = pa.tile([P, 2, HL, 32], bf16, tag="sin", bufs=2, name="sin_t")
                        nc.sync.dma_start(out=sin_t, in_=sin16[s0 : s0 + P, :])

                        pqk = psA.tile([P, 2 * O], f32, tag="pqk", bufs=2, name="pqk")
                        pv = psA.tile([P, O], f32, tag="pv", bufs=2, name="pv")
                        for ec in range(ECH):
                            lhs = xts[ec][:, t * P : (t + 1) * P]
                            st = ec == 0
                            sp = ec == ECH - 1
                            nc.tensor.matmul(pqk[:, 0:O], lhs, wq[ec], start=st, stop=sp)
                            nc.tensor.matmul(pqk[:, O : 2 * O], lhs, wk[ec], start=st, stop=sp)
                            nc.tensor.matmul(pv, lhs, wv[ec], start=st, stop=sp)

                        # V + ones column into persistent v_ones tile
                        nc.vector.memset(vo[bst][:, :, D : D + 1].bitcast(mybir.dt.uint32), 0x3F800000)
                        nc.scalar.copy(
                            out=vo[bst][:, :, 0:D],
                            in_=pv.rearrange("p (h d) -> p h d", h=HL),
                        )

                        # norms are rotation-invariant: square the pre-RoPE
                        # values (ScalarE) and reduce per (s, tensor, head)
                        sq = pa.tile([P, 2 * O], f32, tag="sq", bufs=2, name="sq")
                        nc.scalar.activation(
                            sq, pqk, mybir.ActivationFunctionType.Square
                        )
                        nc.vector.tensor_reduce(
                            out=ssq[:, t, :, :],
                            in_=sq.rearrange("p (u h d) -> p u h d", u=2, h=HL),
                            axis=mybir.AxisListType.X,
                            op=ALU.add,
                        )

                        # RoPE in bf16: cols [h*64, h*64+32) are the 'a'
                        # (even-d) half, [h*64+32, h*64+64) the 'b' (odd-d)
                        # half, for q (cols 0:512) and k (cols 512:1024).
                        qk = pa.tile([P, 2 * O], bf16, tag="qk", bufs=2, name="qk")
                        nc.scalar.copy(out=qk, in_=pqk)
                        qkr = pa.tile([P, 2 * O], bf16, tag="qkr", bufs=4, name="qkr")
                        sv = qk.rearrange("p (u h c) -> p u h c", u=2, h=HL)
                        rv = qkr.rearrange("p (u h c) -> p u h c", u=2, h=HL)
                        a, b = sv[:, :, :, 0:32], sv[:, :, :, 32:64]
                        t1 = pa.tile([P, 2, HL, 32], bf16, tag="rt1", bufs=2, name="rt1")
                        t2 = pa.tile([P, 2, HL, 32], bf16, tag="rt2", bufs=2, name="rt2")
                        nc.vector.tensor_mul(t1, a, cos_t)
                        nc.vector.tensor_mul(t2, b, sin_t)
                        nc.vector.tensor_tensor(
                            out=rv[:, :, :, 0:32], in0=t1, in1=t2, op=ALU.subtract
                        )
                        t3 = pa.tile([P, 2, HL, 32], bf16, tag="rt1", bufs=2, name="rt3")
                        t4 = pa.tile([P, 2, HL, 32], bf16, tag="rt2", bufs=2, name="rt4")
                        nc.vector.tensor_mul(t3, a, sin_t)
                        nc.vector.tensor_mul(t4, b, cos_t)
                        nc.vector.tensor_add(out=rv[:, :, :, 32:64], in0=t3, in1=t4)
                        qkrs.append(qkr)

                    # rsqrt of the block's 4*2*8 sums: bit trick + 2 Newton
                    rsq = pa.tile([P, 4, 2, HL], f32, tag="rsq", bufs=2, name="rsq")
                    yi = pa.tile([P, 4, 2, HL], i32, tag="nwt_i", bufs=2, name="nwt_i")
                    nc.vector.tensor_scalar(
                        out=yi,
                        in0=ssq.bitcast(i32),
                        scalar1=1,
                        scalar2=-1,
                        op0=ALU.logical_shift_right,
                        op1=ALU.bitwise_xor,
                    )
                    nc.vector.tensor_scalar(
                        out=yi, in0=yi, scalar1=_MAGIC_P1, scalar2=None, op0=ALU.add
                    )
                    y = yi.bitcast(f32)
                    for it in range(2):
                        ta = pa.tile([P, 4, 2, HL], f32, tag="nwt_a", bufs=2, name="nwt_a")
                        nc.vector.tensor_mul(ta, y, y)
                        nc.vector.tensor_mul(ta, ta, ssq)
                        nc.vector.tensor_scalar(
                            out=ta,
                            in0=ta,
                            scalar1=-0.5,
                            scalar2=1.5,
                            op0=ALU.mult,
                            op1=ALU.add,
                        )
                        dst = rsq if it == 1 else y
                        nc.vector.tensor_mul(dst, y, ta)

                    # normalize + transpose to [o, s]
                    for t in range(4):
                        bst = blk * 4 + t
                        s0 = bst * P
                        qkr = qkrs[t]
                        nrm = pa.tile([P, 2 * O], sc_dt, tag="nrm", bufs=2, name="nrm")
                        for u in range(2):
                            for h in range(HL):
                                nc.vector.tensor_scalar_mul(
                                    out=nrm[:, u * O + h * D : u * O + (h + 1) * D],
                                    in0=qkr[:, u * O + h * D : u * O + (h + 1) * D],
                                    scalar1=rsq[:, t, u, h : h + 1],
                                )
                        for u, T, scale_col in ((0, qT, None), (1, kT, s2c)):
                            for j in range(OT):
                                ptp = psA.tile([P, P], sc_dt, tag="pt", bufs=2, name="ptp")
                                nc.tensor.transpose(
                                    ptp, nrm[:, u * O + j * P : u * O + (j + 1) * P], ident_b
                                )
                                if scale_col is not None:
                                    nc.scalar.activation(
                                        out=T[j][:, s0 : s0 + P],
                                        in_=ptp,
                                        func=mybir.ActivationFunctionType.Copy,
                                        scale=scale_col[j],
                                    )
                                else:
                                    nc.scalar.copy(
                                        out=T[j][:, s0 : s0 + P], in_=ptp
                                    )

            # ================= phase B: attention =================
            if "b" not in phases:
                pass
            else:
              with tc.tile_pool(name="pb", bufs=1) as pb, tc.tile_pool(
                name="psB", bufs=1, space="PSUM"
            ) as psB:
                for h in range(HL):
                    ot, half = h // 2, h % 2
                    r0 = half * D
                    po = [
                        psB.tile([D + 1, 512], f32, tag="po", bufs=4, name=f"po{qs}")
                        for qs in range(4)
                    ]
                    for kt in range(NKT):
                        lhs_k = kT[ot][r0 : r0 + D, kt * P : (kt + 1) * P]
                        for qh in range(2):
                            sc = psB.tile([P, 1024], f32, tag="sc", bufs=2, name="sc")
                            for qq in range(2):
                                nc.tensor.matmul(
                                    sc[:, qq * 512 : (qq + 1) * 512],
                                    lhs_k,
                                    qT[ot][
                                        r0 : r0 + D,
                                        (qh * 2 + qq) * 512 : (qh * 2 + qq + 1) * 512,
                                    ],
                                    start=True,
                                    stop=True,
                                )
                            e = pb.tile([P, 1024], mm_dt, tag="e", bufs=10, name="e")
                            nc.scalar.activation(
                                e, sc, mybir.ActivationFunctionType.Exp, scale=0.125
                            )
                            for qq in range(2):
                                qs = qh * 2 + qq
                                nc.tensor.matmul(
                                    po[qs],
                                    vo[kt][:, h, :],
                                    e[:, qq * 512 : (qq + 1) * 512],
                                    start=(kt == 0),
                                    stop=(kt == NKT - 1),
                                )

                    for qs in range(4):
                        pvs = pb.tile([D + 1, 512], f32, tag="pvs", bufs=4, name="pvs")
                        nc.vector.tensor_copy(out=pvs, in_=po[qs])
                        for j in range(4):
                            potr = psB.tile([P, D + 1], f32, tag="po", bufs=4, name="potr")
                            nc.tensor.transpose(
                                potr, pvs[:, j * P : (j + 1) * P], ident_f[0 : D + 1, 0 : D + 1]
                            )
                            rec = pb.tile([P, 1], f32, tag="rec", bufs=6, name="rec")
                            nc.vector.reciprocal(rec, potr[:, D : D + 1])
                            qb = qs * 4 + j
                            nc.vector.tensor_scalar_mul(
                                out=out_sb[qb][:, h * D : (h + 1) * D],
                                in0=potr[:, 0:D],
                                scalar1=rec,
                            )

                for qb in range(NBT):
                    nc.sync.dma_start(
                        out=out[qb * P : (qb + 1) * P, :], in_=out_sb[qb]
                    )

    return nc


def shard_inputs(x, Wq, Wk, Wv, sqk, freqs_cos, freqs_sin):
    """Build the 8 per-core input maps (host-side layout prep)."""
    x = np.asarray(x, dtype=np.float32)
    Wq = np.asarray(Wq, dtype=np.float32)
    Wk = np.asarray(Wk, dtype=np.float32)
    Wv = np.asarray(Wv, dtype=np.float32)
    sqk = np.asarray(sqk, dtype=np.float32)
    freqs_cos = np.asarray(freqs_cos, dtype=np.float32)
    freqs_sin = np.asarray(freqs_sin, dtype=np.float32)

    # rope pairing permutation within each head: even d's then odd d's
    perm_local = np.concatenate(
        [h * D + np.concatenate([np.arange(0, D, 2), np.arange(1, D, 2)]) for h in range(HL)]
    )
    s2_full = (sqk * 32.0) ** 2  # (SQK_INIT_VAL / BASE_SCALE) == 32

    import ml_dtypes

    cos16 = np.ascontiguousarray(
        np.tile(freqs_cos, (1, 2 * HL)).astype(ml_dtypes.bfloat16)
    )  # [S, 512] = (q|k) x heads x 32
    sin16 = np.ascontiguousarray(
        np.tile(freqs_sin, (1, 2 * HL)).astype(ml_dtypes.bfloat16)
    )

    xTs = [np.ascontiguousarray(x[b].T) for b in range(B)]

    in_maps = []
    for c in range(NCORES):
        b, hg = c % B, c // B
        rows = hg * O + np.arange(O)
        rows_p = hg * O + perm_local
        in_maps.append(
            {
                "xT": xTs[b],
                "wqT": np.ascontiguousarray(Wq[rows_p, :].T),
                "wkT": np.ascontiguousarray(Wk[rows_p, :].T),
                "wvT": np.ascontiguousarray(Wv[rows, :].T),
                "s2p": np.ascontiguousarray(
                    s2_full[rows_p].reshape(OT, P, 1)
                ),
                "cos16": cos16,
                "sin16": sin16,
            }
        )
    return in_maps


def unshard_output(results):
    """results: list of 8 dicts with 'out' [S, 512] -> full [B, S, E]."""
    full = np.empty((B, S, E), dtype=np.float32)
    for c in range(NCORES):
        b, hg = c % B, c // B
        full[b, :, hg * O : (hg + 1) * O] = results[c]["out"]
    return full


def kernel(x, Wq, Wk, Wv, sqk, freqs_cos, freqs_sin):
    global _built
    _ensure_paths()
    from concourse.bass_utils import run_bass_kernel_spmd

    if _built is None:
        _built = build_program()
    in_maps = shard_inputs(x, Wq, Wk, Wv, sqk, freqs_cos, freqs_sin)
    res = run_bass_kernel_spmd(_built, in_maps, core_ids=list(range(NCORES)))
    return unshard_output(res.results)

